# revision 4
# baseline (speedup 1.0000x reference)
"""TRN2 Bass kernel for nn_CNN_transformer_hr_xyz_41051297415299.

Reference model (B=32, C=512, D=512, H=8, DFF=2048, K=7), per batch element:
    query_in = causal_conv_in(x)                 # conv over last axis t, mixing C
    xn       = LN0(query_in)                     # over t, (x-m)/(std+eps), ddof=1
    q = conv_q(query_in); k = conv_k(xn); v = conv_v(xn)
    heads split the t axis (8 x 64); attention over the C axis
    o  = softmax(q k^T / 8) v   -> (C, D)
    y  = conv_o(o);  h1 = 2y
    hn = LN1(h1)  ==  LN(y) with eps/2
    out = 2 * (relu(hn @ w1 + b1) @ w2 + b2)

Sharding: data-parallel over batch, 4 per NeuronCore, no collectives.
All matmuls run as float32r (TF32-like, full PE rate at N>=256, ~1.5e-4 rel err).

Device layout notes (per batch element b):
    std layout  = [channel c (partitions, 4 chunks), t (free)]
    T   layout  = [t (partitions, 4 chunks), channel (free)]
    x, query_in, xn, o_full : std, padded free dim 6+512 (causal left pad)
    qT, kT  : T (conv emitted transposed: lhsT=activation window, rhs=weight)
    v_aug   : [c (part), chunk, head, 65]  (64 v cols + ones col -> softmax
              denominator accumulates in the same matmul as o = p @ v)
    ff1T    : [dff (16 chunks), c]; ff2 emits std [c, d].
"""
import numpy as np
from contextlib import ExitStack

try:
    import concourse.bass as bass
except ImportError:  # pragma: no cover - path fallback for bare containers
    import sys
    for _p in ("/opt/trn_rl_repo", "/root/.axon_site/_ro/trn_rl_repo"):
        if _p not in sys.path:
            sys.path.insert(0, _p)
    import concourse.bass as bass

import concourse.mybir as mybir
import concourse.tile as tile
from concourse import bacc
from concourse.bass_utils import run_bass_kernel_spmd
from concourse.masks import make_identity

B, C, D, H, DFF, KW = 32, 512, 512, 8, 2048, 7
NCORES = 8
BL = B // NCORES          # 4 batch elements per core
DH = D // H               # 64
PAD = KW - 1              # 6
EPS = 1e-6
F32 = mybir.dt.float32
F32R = mybir.dt.float32r
AF = mybir.ActivationFunctionType
ALU = mybir.AluOpType


def _conv_w_host(w):
    """(cout, cin, KW) -> (4, 128, KW*512): [ci][p][k*512+cout]."""
    return np.ascontiguousarray(
        w.transpose(1, 2, 0).reshape(4, 128, KW * C).astype(np.float32))


def build_nc():
    nc = bacc.Bacc("TRN2", target_bir_lowering=False, debug=False)

    xp = nc.declare_dram_parameter("xp", [BL, 4, 128, PAD + D], F32R, isOutput=False)
    wps = {n: nc.declare_dram_parameter(n, [4, 128, KW * C], F32R, isOutput=False)
           for n in ("win", "wq", "wk", "wv", "wo")}
    w1p = nc.declare_dram_parameter("w1p", [4, 128, DFF], F32R, isOutput=False)
    w2p = nc.declare_dram_parameter("w2p", [16, 128, D], F32R, isOutput=False)
    browp = nc.declare_dram_parameter("browp", [1, 3 * 512], F32R, isOutput=False)   # bq|bk|b2
    onecp = nc.declare_dram_parameter("onecp", [1, 128], F32R, isOutput=False)
    bppp = nc.declare_dram_parameter("bppp", [128, 28], F32, isOutput=False)         # per-part biases
    lnp = {n: nc.declare_dram_parameter(n, [128, D], F32, isOutput=False)
           for n in ("ln0g", "ln0b", "ln1g", "ln1b")}
    onesp = nc.declare_dram_parameter("onesp", [128, 4, 8, 2], F32, isOutput=False)
    zerosp = nc.declare_dram_parameter("zerosp", [128, 4, PAD], F32, isOutput=False)
    outp = nc.declare_dram_parameter("outp", [BL, C, D], F32, isOutput=True)

    with tile.TileContext(nc) as tc, ExitStack() as octx:
        cp = octx.enter_context(tc.tile_pool(name="consts", bufs=1))
        pmm = octx.enter_context(tc.tile_pool(name="pmm", bufs=4, space="PSUM"))
        hpool = octx.enter_context(tc.tile_pool(name="hnTp", bufs=4))

        # ---- constants ----
        def ctile(name, shape, dtype, src):
            t = cp.tile(shape, dtype, tag=name, name=name)
            nc.sync.dma_start(t[:], src)
            return t

        w_t = None  # conv weight tiles, rotated via wconv pool
        brow = ctile("brow", [1, 3 * 512], F32R, browp.ap())
        onec = ctile("onec", [1, 128], F32R, onecp.ap())
        bpp = ctile("bpp", [128, 28], F32, bppp.ap())
        ln_t = {n: ctile(n, [128, D], F32, lnp[n].ap()) for n in lnp}
        ones_t = ctile("ones", [128, 4, 8, 2], F32, onesp.ap())
        zeros_t = ctile("zeros", [128, 4, PAD], F32, zerosp.ap())
        ident = cp.tile([128, 128], F32, tag="ident", name="ident")
        make_identity(nc, ident[:])

        def load_w(pool, param, label):
            ts = []
            for ci in range(4):
                t = pool.tile([128, KW * C], F32R, tag="w", name=f"{label}{ci}")
                nc.sync.dma_start(t[:], param.ap()[ci])
                ts.append(t)
            return ts

        def conv_std(bs, wt, src, writer):
            """std conv: out[cout, t] accumulated over (cin chunk, tap)."""
            for oc in range(4):
                ps = {b: pmm.tile([128, D], F32, tag="mm", name=f"cs{oc}{b}")
                      for b in bs}
                for ci in range(4):
                    for k in range(KW):
                        lhsT = wt[ci][:, k * C + oc * 128: k * C + oc * 128 + 128]
                        for b in bs:
                            nc.tensor.matmul(
                                ps[b][:], lhsT, src[b][:, ci, k:k + D],
                                start=(ci == 0 and k == 0),
                                stop=(ci == 3 and k == KW - 1))
                for b in bs:
                    writer(b, oc, ps[b])

        def conv_T(bs, wt, src, brow_off, dst):
            """transposed conv: out[t, cout]; rank-1 bias matmul first."""
            for tcn in range(4):
                ps = {b: pmm.tile([128, D], F32, tag="mm", name=f"cT{tcn}{b}")
                      for b in bs}
                for b in bs:
                    nc.tensor.matmul(ps[b][:], onec[:],
                                     brow[:, brow_off:brow_off + D],
                                     start=True, stop=False)
                for ci in range(4):
                    for k in range(KW):
                        rhs = wt[ci][:, k * C:(k + 1) * C]
                        for b in bs:
                            lhsT = src[b][:, ci, tcn * 128 + k: tcn * 128 + k + 128]
                            nc.tensor.matmul(ps[b][:], lhsT, rhs, start=False,
                                             stop=(ci == 3 and k == KW - 1))
                for b in bs:
                    nc.vector.tensor_copy(dst[b][:, tcn, :], ps[b][:])

        def emit_ln(bs, lnw, stat, src, dst, g_t, b_t, eps, padded_src):
            for b in bs:
                for c in range(4):
                    sv = (src[b][:, c, PAD:PAD + D] if padded_src
                          else src[b][:, c, :])
                    sv = sv.bitcast(F32) if sv.dtype == F32R else sv
                    sm = stat.tile([128, 1], F32, tag="st", name=f"sm{b}{c}")
                    nc.vector.reduce_sum(sm[:], sv, axis=mybir.AxisListType.X)
                    mn = stat.tile([128, 1], F32, tag="st", name=f"mn{b}{c}")
                    nc.scalar.mul(mn[:], sm[:], 1.0 / D)
                    cent = lnw.tile([128, D], F32, tag="lw", name=f"ce{b}{c}")
                    nc.vector.tensor_scalar(cent[:], sv, mn[:], None,
                                            op0=ALU.subtract)
                    scr = lnw.tile([128, D], F32, tag="lw", name=f"sc{b}{c}")
                    sq = stat.tile([128, 1], F32, tag="st", name=f"sq{b}{c}")
                    nc.scalar.activation(scr[:], cent[:], AF.Square,
                                         accum_out=sq[:])
                    st = stat.tile([128, 1], F32, tag="st", name=f"sd{b}{c}")
                    nc.scalar.activation(st[:], sq[:], AF.Sqrt,
                                         scale=1.0 / (D - 1))
                    dn = stat.tile([128, 1], F32, tag="st", name=f"dn{b}{c}")
                    nc.vector.tensor_scalar_add(dn[:], st[:], eps)
                    iv = stat.tile([128, 1], F32, tag="st", name=f"iv{b}{c}")
                    nc.vector.reciprocal(iv[:], dn[:])
                    tmp = lnw.tile([128, D], F32, tag="lw", name=f"tm{b}{c}")
                    nc.vector.scalar_tensor_tensor(
                        tmp[:], in0=cent[:], scalar=iv[:], in1=g_t[:],
                        op0=ALU.mult, op1=ALU.mult)
                    dv = (dst[b][:, c, PAD:PAD + D] if padded_src
                          else dst[b][:, c, :])
                    nc.vector.tensor_add(dv, tmp[:], b_t[:])

        def zero_pads(t):
            nc.scalar.copy(t[:, :, 0:PAD], zeros_t[:])

        # ======== two passes over batch pairs ========
        with ExitStack() as pctx:
            wconv = pctx.enter_context(tc.tile_pool(name="wconv", bufs=5))
            act = pctx.enter_context(tc.tile_pool(name="act", bufs=8))
            expp = pctx.enter_context(tc.tile_pool(name="expp", bufs=4))
            lnw = pctx.enter_context(tc.tile_pool(name="lnw", bufs=2))
            stat = pctx.enter_context(tc.tile_pool(name="stat", bufs=16))
            patt = pctx.enter_context(
                tc.tile_pool(name="patt", bufs=4, space="PSUM"))
            hnT = {}

            for pi in range(BL // 2):
                bs = [2 * pi, 2 * pi + 1]
                # s1: conv_in
                x_t = {}
                for b in bs:
                    x_t[b] = act.tile([128, 4, PAD + D], F32R, tag="a",
                                      name=f"x{b}")
                    nc.sync.dma_start(x_t[b][:],
                                      xp.ap()[b].rearrange("c p t -> p c t"))
                w_t = load_w(wconv, wps["win"], f"win{pi}")
                qin = {}
                for b in bs:
                    qin[b] = act.tile([128, 4, PAD + D], F32R, tag="a",
                                      name=f"qin{b}")
                    zero_pads(qin[b])

                def wr_qin(b, oc, ps):
                    nc.scalar.activation(qin[b][:, oc, PAD:PAD + D], ps[:],
                                         AF.Identity, bias=bpp[:, oc:oc + 1])
                conv_std(bs, w_t, x_t, wr_qin)

                # s2: LN0
                xn = {}
                for b in bs:
                    xn[b] = act.tile([128, 4, PAD + D], F32R, tag="a",
                                     name=f"xn{b}")
                    zero_pads(xn[b])
                emit_ln(bs, lnw, stat, qin, xn, ln_t["ln0g"], ln_t["ln0b"],
                        EPS, padded_src=True)

                # s3/s4: conv_q (from qin), conv_k (from xn), transposed out
                w_t = load_w(wconv, wps["wq"], f"wq{pi}")
                qT = {b: act.tile([128, 4, D], F32R, tag="a", name=f"qT{b}")
                      for b in bs}
                conv_T(bs, w_t, qin, 0, qT)
                w_t = load_w(wconv, wps["wk"], f"wk{pi}")
                kT = {b: act.tile([128, 4, D], F32R, tag="a", name=f"kT{b}")
                      for b in bs}
                conv_T(bs, w_t, xn, 512, kT)

                # s5: conv_v -> v_aug (std layout + ones col per head)
                w_t = load_w(wconv, wps["wv"], f"wv{pi}")
                vaug = {}
                for b in bs:
                    vaug[b] = act.tile([128, 4, H, DH + 2], F32R, tag="a",
                                       name=f"vaug{b}")
                    nc.scalar.copy(vaug[b][:, :, :, DH:DH + 2], ones_t[:])

                def wr_v(b, oc, ps):
                    nc.scalar.activation(
                        vaug[b][:, oc, :, 0:DH],
                        ps[:].rearrange("p (h dd) -> p h dd", h=H),
                        AF.Identity, bias=bpp[:, 4 + oc:5 + oc])
                conv_std(bs, w_t, xn, wr_v)

                # s6: attention
                ofull = {}
                for b in bs:
                    ofull[b] = act.tile([128, 4, PAD + D], F32R, tag="a",
                                        name=f"of{b}")
                    zero_pads(ofull[b])
                for b in bs:
                    for h in range(H):
                        tcn, prow = h // 2, (h % 2) * DH
                        ops = [patt.tile([128, DH + 2], F32, tag="att",
                                         name=f"o{b}{h}{qc}")
                               for qc in range(4)]
                        for kc in range(4):
                            sp = pmm.tile([128, D], F32, tag="mm",
                                          name=f"s{b}{h}{kc}")
                            nc.tensor.matmul(
                                sp[:],
                                kT[b][prow:prow + DH, tcn, kc * 128:(kc + 1) * 128],
                                qT[b][prow:prow + DH, tcn, :],
                                start=True, stop=True)
                            ex = expp.tile([128, D], F32R, tag="e",
                                           name=f"e{b}{h}{kc}")
                            nc.scalar.activation(ex[:], sp[:], AF.Exp,
                                                 scale=1.0 / np.sqrt(DH))
                            for qc in range(4):
                                nc.tensor.matmul(
                                    ops[qc][:],
                                    ex[:, qc * 128:(qc + 1) * 128],
                                    vaug[b][:, kc, h, :],
                                    start=(kc == 0), stop=(kc == 3))
                        for qc in range(4):
                            rec = stat.tile([128, 1], F32, tag="st",
                                            name=f"r{b}{h}{qc}")
                            nc.vector.reciprocal(rec[:], ops[qc][:, DH:DH + 1])
                            nc.vector.tensor_scalar_mul(
                                ofull[b][:, qc, PAD + h * DH: PAD + (h + 1) * DH],
                                ops[qc][:, 0:DH], rec[:])

                # s7: conv_o -> y (fp32)
                w_t = load_w(wconv, wps["wo"], f"wo{pi}")
                y = {b: act.tile([128, 4, D], F32, tag="a", name=f"y{b}")
                     for b in bs}

                def wr_y(b, oc, ps):
                    nc.scalar.activation(y[b][:, oc, :], ps[:], AF.Identity,
                                         bias=bpp[:, 8 + oc:9 + oc])
                conv_std(bs, w_t, ofull, wr_y)

                # s8: LN1 (eps/2 absorbs h1 = 2y), then transpose -> hnT
                hn = {b: act.tile([128, 4, D], F32, tag="a", name=f"hn{b}")
                      for b in bs}
                emit_ln(bs, lnw, stat, y, hn, ln_t["ln1g"], ln_t["ln1b"],
                        EPS / 2, padded_src=False)
                for b in bs:
                    hnT[b] = hpool.tile([128, 4, D], F32R, tag="h",
                                        name=f"hnT{b}")
                    for tcn in range(4):
                        for cc in range(4):
                            tp = patt.tile([128, 128], F32, tag="att",
                                           name=f"tp{b}{tcn}{cc}")
                            nc.tensor.transpose(
                                tp[:], hn[b][:, cc, tcn * 128:(tcn + 1) * 128],
                                ident[:])
                            nc.vector.tensor_copy(
                                hnT[b][:, tcn, cc * 128:(cc + 1) * 128], tp[:])

        # ======== FFN phase (all 4 b) ========
        with ExitStack() as fctx:
            w1pool = fctx.enter_context(tc.tile_pool(name="w1pool", bufs=4))
            w2pool = fctx.enter_context(tc.tile_pool(name="w2pool", bufs=16))
            rpool = fctx.enter_context(tc.tile_pool(name="rpool", bufs=2))
            obp = fctx.enter_context(tc.tile_pool(name="obp", bufs=4))
            stat2 = fctx.enter_context(tc.tile_pool(name="stat2", bufs=8))
            pff = fctx.enter_context(
                tc.tile_pool(name="pff", bufs=4, space="PSUM"))

            w1t = []
            for tcn in range(4):
                t = w1pool.tile([128, DFF], F32R, tag="w1", name=f"w1_{tcn}")
                nc.sync.dma_start(t[:], w1p.ap()[tcn])
                w1t.append(t)
            w2t = []
            for fc in range(16):
                t = w2pool.tile([128, D], F32R, tag="w2", name=f"w2_{fc}")
                nc.sync.dma_start(t[:], w2p.ap()[fc])
                w2t.append(t)

            for pi in range(BL // 2):
                bs = [2 * pi, 2 * pi + 1]
                rl = {b: rpool.tile([128, 16, D], F32R, tag="r", name=f"rl{b}")
                      for b in bs}
                for fc in range(16):
                    ps = {b: pff.tile([128, D], F32, tag="f1", name=f"f{fc}{b}")
                          for b in bs}
                    for tcn in range(4):
                        lhsT = w1t[tcn][:, fc * 128:(fc + 1) * 128]
                        for b in bs:
                            nc.tensor.matmul(ps[b][:], lhsT,
                                             hnT[b][:, tcn, :],
                                             start=(tcn == 0), stop=(tcn == 3))
                    for b in bs:
                        nc.scalar.activation(rl[b][:, fc, :], ps[b][:], AF.Relu,
                                             bias=bpp[:, 12 + fc:13 + fc])
                for cc in range(4):
                    ps2 = {b: pmm.tile([128, D], F32, tag="mm",
                                       name=f"g{cc}{b}") for b in bs}
                    for b in bs:
                        nc.tensor.matmul(ps2[b][:], onec[:],
                                         brow[:, 1024:1024 + D],
                                         start=True, stop=False)
                    for fc in range(16):
                        rhs = w2t[fc][:]
                        for b in bs:
                            nc.tensor.matmul(
                                ps2[b][:],
                                rl[b][:, fc, cc * 128:(cc + 1) * 128], rhs,
                                start=False, stop=(fc == 15))
                    for b in bs:
                        ob = obp.tile([128, D], F32, tag="ob", name=f"ob{cc}{b}")
                        nc.scalar.activation(ob[:], ps2[b][:], AF.Copy,
                                             scale=2.0)
                        nc.sync.dma_start(
                            outp.ap()[b, cc * 128:(cc + 1) * 128, :], ob[:])

    nc.compile()
    return nc


def prep_in_maps(inputs):
    """Full inputs -> list of 8 per-core input dicts (host-side prep)."""
    f = lambda a: np.ascontiguousarray(np.asarray(a, dtype=np.float32))
    x = f(inputs["x"])
    xpad = np.zeros((B, 4, 128, PAD + D), np.float32)
    xpad[:, :, :, PAD:] = x.reshape(B, 4, 128, D)

    shared = {
        "win": _conv_w_host(f(inputs["w_conv_in"])),
        "wq": _conv_w_host(f(inputs["wq"])),
        "wk": _conv_w_host(f(inputs["wk"])),
        "wv": _conv_w_host(f(inputs["wv"])),
        "wo": _conv_w_host(f(inputs["wo"])),
        "w1p": f(inputs["w1"]).reshape(4, 128, DFF),
        "w2p": f(inputs["w2"]).reshape(16, 128, D),
        "browp": np.concatenate(
            [f(inputs["bq"]), f(inputs["bk"]), f(inputs["b2"])])[None, :],
        "onecp": np.ones((1, 128), np.float32),
        "bppp": np.stack(
            [f(inputs["b_conv_in"]).reshape(4, 128)[i] for i in range(4)]
            + [f(inputs["bv"]).reshape(4, 128)[i] for i in range(4)]
            + [f(inputs["bo"]).reshape(4, 128)[i] for i in range(4)]
            + [f(inputs["b1"]).reshape(16, 128)[i] for i in range(16)],
            axis=1),
        "ln0g": np.tile(f(inputs["ln0_g"]), (128, 1)),
        "ln0b": np.tile(f(inputs["ln0_b"]), (128, 1)),
        "ln1g": np.tile(f(inputs["ln1_g"]), (128, 1)),
        "ln1b": np.tile(f(inputs["ln1_b"]), (128, 1)),
        "onesp": np.concatenate([np.ones((128, 4, 8, 1), np.float32),
                                 np.zeros((128, 4, 8, 1), np.float32)], axis=3),
        "zerosp": np.zeros((128, 4, PAD), np.float32),
    }
    shared = {k: np.ascontiguousarray(v) for k, v in shared.items()}
    return [dict(shared, xp=np.ascontiguousarray(xpad[c * BL:(c + 1) * BL]))
            for c in range(NCORES)]


_NC_CACHE = None


def get_nc():
    global _NC_CACHE
    if _NC_CACHE is None:
        _NC_CACHE = build_nc()
    return _NC_CACHE


def kernel(**inputs) -> np.ndarray:
    nc = get_nc()
    in_maps = prep_in_maps(inputs)
    res = run_bass_kernel_spmd(nc, in_maps, list(range(NCORES)))
    return np.concatenate([res.results[c]["outp"] for c in range(NCORES)],
                          axis=0).astype(np.float32)
